# revision 19
# baseline (speedup 1.0000x reference)
"""ColourCatDSSGINConv on 8 trn2 NeuronCores.

Sharding: nodes are partitioned into 8 contiguous blocks of 6250; each core
aggregates the in-edges of its own nodes (pull model) from a replicated
node-feature table U = [x | c] (48 cols, padded to 64), then runs both GIN
MLP paths on its block.

Aggregation: per-phase (src<32768 / src>=32768, int16 gather indices) ELL
iterations over degree-sorted nodes; dense-prefix dma_gather calls (2048
rows, 256B each) round-robined over 4 SWDGE queues so descriptor
generation runs on all 8 Q7 cores concurrently; DVE adds accumulate into
SBUF.  The hi-phase accumulator is merged into canonical (lo) order with
dma_scatter_add into a DRAM table.

The epilogue is pipelined INTO the lo gather stream: as soon as the last
ELL iteration touching a 512-node chunk retires, that chunk's stacked
features are built, transposed, and pushed through the W1 matmul; BN
statistics (colsum + second-moment matmul) accumulate per chunk.  After
the last chunk: stats fold -> AllReduce[64,4] -> BN coefficients -> pass 2
(BN+relu from stored bf16 h1, W2 matmuls, output DMA).  b1s/b1a cancel
inside BatchNorm and are dropped.  Output is feature-major [256, 6250]
per core; the host transposes.
"""
import os
import sys

sys.path.insert(0, "/opt/trn_rl_repo")

import numpy as np

N = 50000
E = 800000
IN = 16
CD = 8
S = 4
EMB = 64
D = IN + CD          # 24
H = 64
BN_EPS = 1e-5

NCORES = 8
P = 128
PC = N // NCORES     # 6250 nodes per core
Q = (PC + P - 1) // P          # 49 column-groups of 128 nodes
SLOTS = Q * P                  # 6272
TROWS = 50176                  # 1 zero row + 50000 nodes + zero pad
LO_ROWS = 32768                # lo window rows [0, 32768): node v at row v+1
HI_BASE = 32768                # hi window: node v (>=32767) at row v+1
HI_ZERO = 50001 - HI_BASE      # a guaranteed-zero row in the hi window
LO_MAX_NODE = 32766
CALL = 1024                    # idxs per dma_gather call
SCALL = 1024                   # idxs per dma_scatter_add call
NQ = 4                         # swdge queues

last_exec_time_ns = None
_prog_cache = {}
_trace = bool(os.environ.get("GNN_TRACE"))


def _wrap16(flat, pad_val, pad_to):
    """int16 flat token list -> [128, pad_to//16] wrapped+replicated layout
    (token t lives at [t%16, t//16], replicated across the 8 gpsimd groups)."""
    n = len(flat)
    assert pad_to % 16 == 0 and n <= pad_to
    buf = np.full(pad_to, pad_val, dtype=np.int16)
    buf[:n] = flat
    arr = buf.reshape(pad_to // 16, 16).T.copy()
    return np.tile(arr, (8, 1))


def _phase_prep(src, ldst, lo):
    """ELL prep for one (core, phase): degree-desc node order; every edge gets
    (slot t, iteration j, int16 table idx)."""
    deg = np.bincount(ldst, minlength=PC)
    order = np.argsort(-deg, kind="stable")
    rank = np.empty(PC, dtype=np.int64)
    rank[order] = np.arange(PC)
    sidx = np.argsort(ldst, kind="stable")
    sd = ldst[sidx]
    ss = src[sidx]
    starts = np.searchsorted(sd, np.arange(PC))
    j = np.arange(len(sd)) - starts[sd]
    t = rank[sd]
    val = (ss + 1 - (0 if lo else HI_BASE)).astype(np.int16)
    return deg[order], order, t, j, val


def _pieces(n_pad, call):
    """Per gather-call DVE-add pieces: [(stg_j0, nj, acc_q0), ...] per call."""
    offs = [0]
    for n in n_pad:
        offs.append(offs[-1] + n)
    L = offs[-1]
    Lpad = ((L + call - 1) // call) * call
    out = []
    for c0 in range(0, Lpad, call):
        c1 = c0 + call
        ps = []
        for j, n in enumerate(n_pad):
            a, b = max(c0, offs[j]), min(c1, offs[j + 1])
            if a < b:
                ps.append(((a - c0) // P, (b - a) // P, (a - offs[j]) // P))
        out.append(ps)
    return out, Lpad


def _build_inputs(x, c, edge_index, eps_a, W1s, g1s, be1s, W2s, b2s,
                  W1a, g1a, be1a, W2a, b2a):
    src_all = edge_index[0].astype(np.int64)
    dst_all = edge_index[1].astype(np.int64)

    U = np.zeros((TROWS, 64), dtype=np.float32)
    U[1:N + 1, :IN] = x
    U[1:N + 1, IN:48] = c.reshape(N, S * CD)

    core_of = dst_all // PC
    meta = {}
    idx_arrays = {}
    scat_arrays = {}
    per = {}
    for k in range(NCORES):
        m = core_of == k
        s_k = src_all[m]
        d_k = dst_all[m] % PC
        lo_m = s_k <= LO_MAX_NODE
        per[(k, "lo")] = _phase_prep(s_k[lo_m], d_k[lo_m], True)
        per[(k, "hi")] = _phase_prep(s_k[~lo_m], d_k[~lo_m], False)

    for ph in ("lo", "hi"):
        maxdeg = max(int(per[(k, ph)][0][0]) if per[(k, ph)][0].size else 0
                     for k in range(NCORES))
        n_pad = []
        for j in range(maxdeg):
            nj = max(int(np.sum(per[(k, ph)][0] > j)) for k in range(NCORES))
            n_pad.append(((nj + P - 1) // P) * P)
        offs = np.concatenate([[0], np.cumsum(n_pad)]).astype(np.int64)
        _, Lpad = _pieces(n_pad, CALL)
        zero_idx = 0 if ph == "lo" else HI_ZERO
        meta[ph] = tuple(n_pad)
        for k in range(NCORES):
            _deg, order, t, j, val = per[(k, ph)]
            flat = np.full(Lpad, zero_idx, dtype=np.int16)
            flat[offs[j] + t] = val
            idx_arrays[(k, ph)] = _wrap16(flat, zero_idx, Lpad)
    # canonical per-core slot order = lo-phase order; only hi needs a merge
    orders = {}
    for k in range(NCORES):
        order_lo = per[(k, "lo")][1]
        rank_lo = np.empty(PC, dtype=np.int64)
        rank_lo[order_lo] = np.arange(PC)
        order_hi = per[(k, "hi")][1]
        ids = np.concatenate([rank_lo[order_hi],
                              np.arange(PC, SLOTS)]).astype(np.int16)
        scat_arrays[(k, "hi")] = _wrap16(ids, 0, SLOTS)
        orders[k] = order_lo

    import ml_dtypes
    wall = np.zeros((128, 320), dtype=np.float32)
    for s in range(S):
        wall[0:IN, s * H:(s + 1) * H] = W1s[0:IN, :]
        wall[IN + CD * s:IN + CD * (s + 1), s * H:(s + 1) * H] = W1s[IN:D, :]
    wall[64:64 + IN, 256:320] = W1a[0:IN, :]
    wall[64 + IN:64 + D, 256:320] = W1a[IN:D, :]
    w2 = np.concatenate([W2s, W2a], axis=1).astype(ml_dtypes.bfloat16)

    bnc = np.zeros((64, 6), dtype=np.float32)
    bnc[:, 0] = g1s
    bnc[:, 1] = be1s
    bnc[:, 2] = 1.0 / (N * S)
    bnc[:, 3] = g1a
    bnc[:, 4] = be1a
    bnc[:, 5] = 1.0 / N
    b2v = (b2s + b2a).astype(np.float32).reshape(64, 1)
    wallT = np.zeros((64, 640), dtype=np.float32)
    for b in range(5):
        wallT[:, b * 128:(b + 1) * 128] = wall[:, b * 64:(b + 1) * 64].T

    in_maps = []
    for k in range(NCORES):
        # own-side contribution to the stacked features, node-major
        uo = np.zeros((SLOTS, 128), dtype=np.float32)
        rows = U[1 + k * PC + orders[k]]
        uo[:PC, 0:48] = (1.0 + 0.0) * rows[:, 0:48]          # scaled below
        in_maps.append({"rows": rows, "uo": uo})

    ret_maps = []
    for k in range(NCORES):
        ret_maps.append({
            "utab": U,
            "ilo": idx_arrays[(k, "lo")],
            "ihi": idx_arrays[(k, "hi")],
            "shi": scat_arrays[(k, "hi")],
            "wall": wall.astype(ml_dtypes.bfloat16),
            "w2": w2,
            "bnc": bnc,
            "b2v": b2v,
            "wallf": wall,
            "wallT": wallT,
        })
    return ret_maps, in_maps, meta, orders


def _finish_inputs(ret_maps, pre_maps, eps_s, eps_a):
    for k in range(NCORES):
        rows = pre_maps[k]["rows"]
        uo = pre_maps[k]["uo"]
        uo[:PC, 0:48] = (1.0 + eps_s) * rows[:, 0:48]
        uo[:PC, 64:80] = (1.0 + eps_a) * rows[:, 0:IN]
        cmean = rows[:, IN:48].reshape(PC, S, CD).mean(axis=1)
        uo[:PC, 80:88] = (1.0 + eps_a) * cmean
        uo_t = uo.reshape(Q, P, 128).transpose(1, 0, 2).reshape(
            P, Q * 128).copy()
        ret_maps[k]["uown2"] = uo_t
    return ret_maps


def _build_program(meta, eps_s, eps_a, clo, chi):
    import concourse.bacc as bacc
    import concourse.tile as tile
    import concourse.mybir as mybir
    from concourse.masks import make_identity

    f32 = mybir.dt.float32
    bf16 = mybir.dt.bfloat16
    i16 = mybir.dt.int16
    add = mybir.AluOpType.add
    sub = mybir.AluOpType.subtract
    mult = mybir.AluOpType.mult
    Relu = mybir.ActivationFunctionType.Relu

    nc = bacc.Bacc("TRN2", target_bir_lowering=False, debug=False,
                   num_devices=NCORES, num_swdge_queues=NQ)
    utab = nc.dram_tensor("utab", [TROWS, 64], f32, kind="ExternalInput").ap()
    uown2 = nc.dram_tensor("uown2", [P, Q * 128], f32,
                           kind="ExternalInput").ap()
    ilo = nc.dram_tensor("ilo", [128, clo // 16], i16, kind="ExternalInput").ap()
    ihi = nc.dram_tensor("ihi", [128, chi // 16], i16, kind="ExternalInput").ap()
    shi = nc.dram_tensor("shi", [128, SLOTS // 16], i16, kind="ExternalInput").ap()
    wallp = nc.dram_tensor("wall", [128, 320], bf16, kind="ExternalInput").ap()
    w2p = nc.dram_tensor("w2", [64, 128], bf16, kind="ExternalInput").ap()
    bncp = nc.dram_tensor("bnc", [64, 6], f32, kind="ExternalInput").ap()
    b2vp = nc.dram_tensor("b2v", [64, 1], f32, kind="ExternalInput").ap()
    wallfp = nc.dram_tensor("wallf", [128, 320], f32, kind="ExternalInput").ap()
    wallTp = nc.dram_tensor("wallT", [64, 640], f32, kind="ExternalInput").ap()
    o_ap = nc.dram_tensor("o", [256, PC], f32, kind="ExternalOutput").ap()

    # chunks of up to 4 column-groups (512 slots)
    chunks = [(q0, min(4, Q - q0)) for q0 in range(0, Q, 4)]
    pieces_lo, _ = _pieces(list(meta["lo"]), CALL)
    pieces_hi, _ = _pieces(list(meta["hi"]), CALL)
    last_call_q = [0] * Q
    for ci, ps in enumerate(pieces_lo):
        for (_sj, nj, qa) in ps:
            for q in range(qa, min(qa + nj, Q)):
                last_call_q[q] = ci
    ready_at = {}
    for gi, (q0, nq) in enumerate(chunks):
        rc = max(last_call_q[q0:q0 + nq])
        ready_at.setdefault(rc, []).append(gi)

    with tile.TileContext(nc) as tc:
        with (
            tc.tile_pool(name="dram", bufs=1, space="DRAM") as dp,
            tc.tile_pool(name="persist", bufs=1) as mp,
        ):
            accd = dp.tile([SLOTS, 64], f32)

            # ---------- persistent tiles (idx tables first: gathers wait) ----
            it_hi = mp.tile([128, chi // 16], i16, tag="it_hi")
            nc.sync.dma_start(out=it_hi[:], in_=ihi[:])
            it_lo = mp.tile([128, clo // 16], i16, tag="it_lo")
            nc.sync.dma_start(out=it_lo[:], in_=ilo[:])
            st = mp.tile([128, SLOTS // 16], i16, tag="st_hi")
            nc.sync.dma_start(out=st[:], in_=shi[:])
            wt = mp.tile([P, 320], bf16, tag="wt")
            nc.sync.dma_start(out=wt[:], in_=wallp[:])
            wtf = mp.tile([P, 320], f32, tag="wtf")
            nc.sync.dma_start(out=wtf[:], in_=wallfp[:])
            wtT = mp.tile([64, 640], f32, tag="wtT")
            nc.sync.dma_start(out=wtT[:], in_=wallTp[:])
            w2t = mp.tile([64, 128], bf16, tag="w2t")
            nc.sync.dma_start(out=w2t[:], in_=w2p[:])
            bnct = mp.tile([64, 6], f32, tag="bnct")
            nc.sync.dma_start(out=bnct[:], in_=bncp[:])
            b2t = mp.tile([64, 1], f32, tag="b2t")
            nc.sync.dma_start(out=b2t[:], in_=b2vp[:])
            ident = mp.tile([P, P], bf16, tag="ident")
            make_identity(nc, ident[:])
            own2 = mp.tile([P, Q, 128], f32, tag="own2")
            nc.sync.dma_start(out=own2[:].rearrange("p q d -> p (q d)"),
                              in_=uown2[:])

            acc_lo = mp.tile([P, Q, 64], f32, tag="acc_lo")
            nc.vector.memset(acc_lo[:].rearrange("p q d -> p (q d)"), 0.0)
            acc_hi = mp.tile([P, Q, 64], f32, tag="acc_hi")
            nc.vector.memset(acc_hi[:].rearrange("p q d -> p (q d)"), 0.0)
            csum = mp.tile([P, 1], f32, tag="csum")
            nc.vector.memset(csum[:], 0.0)
            hA = mp.tile([P, SLOTS], bf16, tag="hA")
            hB = mp.tile([P, SLOTS], bf16, tag="hB")
            hC = mp.tile([64, SLOTS], bf16, tag="hC")
            stats = mp.tile([64, 4], f32, tag="stats")
            m2s = mp.tile([P, P], f32, tag="m2s")

            qrot = [0]

            def rr():
                qn = qrot[0] % NQ
                qrot[0] += 1
                return qn

            # ---------- phase 1: gather + ELL accumulate + chunk epilogue ---
            with (
                tc.tile_pool(name="stg", bufs=8) as sp,
                tc.tile_pool(name="zp", bufs=1) as zp,
                tc.tile_pool(name="accmp", bufs=3) as accmp,
                tc.tile_pool(name="stkp", bufs=3) as stkp,
                tc.tile_pool(name="ptp", bufs=1, space="PSUM") as ptp,
                tc.tile_pool(name="pw1", bufs=2, space="PSUM") as pw1,
                tc.tile_pool(name="pm2m", bufs=1, space="PSUM") as pm2m,
            ):
                zt = zp.tile([P, Q * 64], f32, tag="zt")
                nc.vector.memset(zt[:], 0.0)
                nc.sync.dma_start(
                    out=accd[:].rearrange("(q p) d -> p q d", p=P),
                    in_=zt[:].rearrange("p (q d) -> p q d", d=64))

                m2 = pm2m.tile([P, P], f32, tag="m2", space="PSUM")
                m2_cnt = [0]

                def emit_chunk(gi):
                    q0, nq = chunks[gi]
                    cw = nq * P
                    c0 = q0 * P
                    accm = accmp.tile([P, 4, 64], f32, tag="accm",
                                      name=f"accm_{gi}")
                    nc.sync.dma_start(
                        out=accm[:, 0:nq, :],
                        in_=accd[c0:c0 + cw, :].rearrange(
                            "(q p) d -> p q d", p=P))
                    stk = stkp.tile([P, 4, 128], f32, tag="stk",
                                    name=f"stk_{gi}")
                    nc.sync.dma_start(
                        out=stk[:, 0:nq, :].rearrange("p q d -> p (q d)"),
                        in_=uown2[:, q0 * 128:(q0 + nq) * 128])
                    # shared msg cols 0:48 (u-agg cols 0:48); wall reads 0:64
                    nc.vector.tensor_tensor(
                        out=stk[:, 0:nq, 0:48], in0=stk[:, 0:nq, 0:48],
                        in1=accm[:, 0:nq, 0:48], op=add)
                    nc.vector.tensor_tensor(
                        out=stk[:, 0:nq, 0:48], in0=stk[:, 0:nq, 0:48],
                        in1=acc_lo[:, q0:q0 + nq, 0:48], op=add)
                    # agg-path x part cols 64:80
                    nc.vector.tensor_tensor(
                        out=stk[:, 0:nq, 64:80], in0=stk[:, 0:nq, 64:80],
                        in1=accm[:, 0:nq, 0:16], op=add)
                    nc.vector.tensor_tensor(
                        out=stk[:, 0:nq, 64:80], in0=stk[:, 0:nq, 64:80],
                        in1=acc_lo[:, q0:q0 + nq, 0:16], op=add)
                    # agg-path c part cols 80:88: own + 0.25*sum_s agg_c_s
                    tcc = stkp.tile([P, 4, 32], f32, tag="tcc",
                                    name=f"tcc_{gi}")
                    nc.vector.tensor_tensor(
                        out=tcc[:, 0:nq, :], in0=accm[:, 0:nq, 16:48],
                        in1=acc_lo[:, q0:q0 + nq, 16:48], op=add)
                    nc.vector.tensor_tensor(
                        out=tcc[:, 0:nq, 0:8], in0=tcc[:, 0:nq, 0:8],
                        in1=tcc[:, 0:nq, 8:16], op=add)
                    nc.vector.tensor_tensor(
                        out=tcc[:, 0:nq, 16:24], in0=tcc[:, 0:nq, 16:24],
                        in1=tcc[:, 0:nq, 24:32], op=add)
                    nc.vector.tensor_tensor(
                        out=tcc[:, 0:nq, 0:8], in0=tcc[:, 0:nq, 0:8],
                        in1=tcc[:, 0:nq, 16:24], op=add)
                    nc.vector.scalar_tensor_tensor(
                        out=stk[:, 0:nq, 80:88], in0=tcc[:, 0:nq, 0:8],
                        scalar=0.25, op0=mult,
                        in1=stk[:, 0:nq, 80:88], op1=add)
                    # bf16 + transpose to feature-major
                    stkb = stkp.tile([P, 4, 128], bf16, tag="stkb",
                                     name=f"stkb_{gi}")
                    nc.vector.tensor_copy(
                        out=stkb[:, 0:nq, :].rearrange("p q d -> p (q d)"),
                        in_=stk[:, 0:nq, :].rearrange("p q d -> p (q d)"))
                    tp = ptp.tile([P, 4, P], bf16, tag="tp", name=f"tp_{gi}",
                                  space="PSUM")
                    for qq in range(nq):
                        nc.tensor.transpose(out=tp[:, qq, :],
                                            in_=stkb[:, qq, :],
                                            identity=ident[:])
                        nc.tensor.matmul(out=m2[:], lhsT=stkb[:, qq, :],
                                         rhs=stkb[:, qq, :],
                                         start=(m2_cnt[0] == 0),
                                         stop=(m2_cnt[0] == Q - 1))
                        m2_cnt[0] += 1
                    stT = stkp.tile([P, 512], bf16, tag="stT",
                                    name=f"stT_{gi}")
                    nc.vector.tensor_copy(
                        out=stT[:, 0:cw],
                        in_=tp[:, 0:nq, :].rearrange("p q d -> p (q d)"))
                    # colsum accumulate (over stacked features)
                    redS = stkp.tile([P, 1], f32, tag="redS",
                                     name=f"redS_{gi}")
                    nc.vector.tensor_reduce(out=redS[:], in_=stT[:, 0:cw],
                                            axis=mybir.AxisListType.X, op=add)
                    nc.vector.tensor_tensor(out=csum[:], in0=csum[:],
                                            in1=redS[:], op=add)
                    # W1 matmuls -> h1 (bf16, no BN yet)
                    psA = pw1.tile([P, 512], f32, tag="psA",
                                   name=f"psA_{gi}", space="PSUM")
                    nc.tensor.matmul(out=psA[:, 0:cw], lhsT=wt[:, 0:128],
                                     rhs=stT[:, 0:cw], start=True, stop=True)
                    nc.scalar.copy(out=hA[:, c0:c0 + cw], in_=psA[:, 0:cw])
                    psB = pw1.tile([P, 512], f32, tag="psB",
                                   name=f"psB_{gi}", space="PSUM")
                    nc.tensor.matmul(out=psB[:, 0:cw], lhsT=wt[:, 128:256],
                                     rhs=stT[:, 0:cw], start=True, stop=True)
                    nc.scalar.copy(out=hB[:, c0:c0 + cw], in_=psB[:, 0:cw])
                    psC = pw1.tile([64, 512], f32, tag="psC",
                                   name=f"psC_{gi}", space="PSUM")
                    nc.tensor.matmul(out=psC[:, 0:cw], lhsT=wt[:, 256:320],
                                     rhs=stT[:, 0:cw], start=True, stop=True)
                    nc.scalar.copy(out=hC[:, c0:c0 + cw], in_=psC[:, 0:cw])

                # --- hi phase: gathers + adds, then scatter-merge ---
                for ci, ps in enumerate(pieces_hi):
                    g = sp.tile([P, CALL // P, 64], f32, tag="stg",
                                name=f"g_hi_{ci}")
                    nc.gpsimd.dma_gather(
                        out_ap=g[:], in_ap=utab[HI_BASE:TROWS, :],
                        idxs_ap=it_hi[:, ci * (CALL // 16):
                                      (ci + 1) * (CALL // 16)],
                        num_idxs=CALL, num_idxs_reg=CALL, elem_size=64,
                        queue_num=rr())
                    for (sj, nj, qa) in ps:
                        nc.vector.tensor_tensor(
                            out=acc_hi[:, qa:qa + nj, :],
                            in0=acc_hi[:, qa:qa + nj, :],
                            in1=g[:, sj:sj + nj, :], op=add)
                for si, c0 in enumerate(range(0, SLOTS, SCALL)):
                    n = min(SCALL, SLOTS - c0)
                    nc.gpsimd.dma_scatter_add(
                        accd[:], acc_hi[:, c0 // P:(c0 + n) // P, :],
                        st[:, c0 // 16:(c0 + n) // 16],
                        num_idxs=n, num_idxs_reg=n, elem_size=64,
                        queue_num=rr())

                # --- lo phase with interleaved chunk work ---
                for ci, ps in enumerate(pieces_lo):
                    g = sp.tile([P, CALL // P, 64], f32, tag="stg",
                                name=f"g_lo_{ci}")
                    nc.gpsimd.dma_gather(
                        out_ap=g[:], in_ap=utab[0:LO_ROWS, :],
                        idxs_ap=it_lo[:, ci * (CALL // 16):
                                      (ci + 1) * (CALL // 16)],
                        num_idxs=CALL, num_idxs_reg=CALL, elem_size=64,
                        queue_num=rr())
                    for (sj, nj, qa) in ps:
                        nc.vector.tensor_tensor(
                            out=acc_lo[:, qa:qa + nj, :],
                            in0=acc_lo[:, qa:qa + nj, :],
                            in1=g[:, sj:sj + nj, :], op=add)
                    for gi in ready_at.get(ci, []):
                        emit_chunk(gi)
                nc.vector.tensor_copy(out=m2s[:], in_=m2[:])

            # ---------- stats fold + AllReduce + BN + pass 2 ----------
            cin = dp.tile([64, 4], f32)
            cout = dp.tile([64, 4], f32)
            with (
                tc.tile_pool(name="po", bufs=2, space="PSUM") as po,
                tc.tile_pool(name="pst", bufs=1, space="PSUM") as pst,
                tc.tile_pool(name="rts", bufs=1) as rts,
                tc.tile_pool(name="outs", bufs=3) as osb,
                tc.tile_pool(name="bnp", bufs=1) as bnp,
            ):
                # s1 (sum of z) per 64-block via wall^T @ colsum;
                # s2 (sum of z^2) via w^T M2 w per block
                s1 = bnp.tile([64, 2], f32, tag="s1")
                s2 = bnp.tile([64, 2], f32, tag="s2")
                nc.vector.memset(s1[:], 0.0)
                nc.vector.memset(s2[:], 0.0)
                for b in range(5):
                    col = 0 if b < 4 else 1
                    wm = pst.tile([64, P], f32, tag="wm", name=f"wm_{b}",
                                  space="PSUM")
                    nc.tensor.matmul(out=wm[:], lhsT=wtf[:, b * 64:(b + 1) * 64],
                                     rhs=m2s[:], start=True, stop=True)
                    prod = bnp.tile([64, P], f32, tag="prod", name=f"prod_{b}")
                    nc.vector.tensor_tensor(
                        out=prod[:], in0=wm[:],
                        in1=wtT[:, b * 128:(b + 1) * 128], op=mult)
                    r2 = bnp.tile([64, 1], f32, tag="r2", name=f"r2_{b}")
                    nc.vector.tensor_reduce(out=r2[:], in_=prod[:],
                                            axis=mybir.AxisListType.X, op=add)
                    nc.vector.tensor_tensor(
                        out=s2[:, col:col + 1], in0=s2[:, col:col + 1],
                        in1=r2[:], op=add)
                    p1 = pst.tile([64, 1], f32, tag="p1", name=f"p1_{b}",
                                  space="PSUM")
                    nc.tensor.matmul(out=p1[:], lhsT=wtf[:, b * 64:(b + 1) * 64],
                                     rhs=csum[:], start=True, stop=True)
                    r1 = bnp.tile([64, 1], f32, tag="r1", name=f"r1_{b}")
                    nc.vector.tensor_copy(out=r1[:], in_=p1[:])
                    nc.vector.tensor_tensor(
                        out=s1[:, col:col + 1], in0=s1[:, col:col + 1],
                        in1=r1[:], op=add)
                nc.vector.tensor_copy(out=stats[:, 0:2], in_=s1[:])
                nc.vector.tensor_copy(out=stats[:, 2:4], in_=s2[:])

                nc.sync.dma_start(out=cin[:], in_=stats[:])
                nc.gpsimd.collective_compute(
                    "AllReduce", add,
                    replica_groups=[list(range(NCORES))],
                    ins=[cin.opt()], outs=[cout.opt()])
                nc.sync.dma_start(out=stats[:], in_=cout[:])

                gcols = bnp.tile([64, 2], f32, tag="gcols")
                becols = bnp.tile([64, 2], f32, tag="becols")
                incols = bnp.tile([64, 2], f32, tag="incols")
                nc.vector.tensor_copy(out=gcols[:, 0:1], in_=bnct[:, 0:1])
                nc.vector.tensor_copy(out=gcols[:, 1:2], in_=bnct[:, 3:4])
                nc.vector.tensor_copy(out=becols[:, 0:1], in_=bnct[:, 1:2])
                nc.vector.tensor_copy(out=becols[:, 1:2], in_=bnct[:, 4:5])
                nc.vector.tensor_copy(out=incols[:, 0:1], in_=bnct[:, 2:3])
                nc.vector.tensor_copy(out=incols[:, 1:2], in_=bnct[:, 5:6])
                mu = bnp.tile([64, 2], f32, tag="mu")
                var = bnp.tile([64, 2], f32, tag="var")
                scl = bnp.tile([64, 2], f32, tag="scl")
                bia = bnp.tile([64, 2], f32, tag="bia")
                nc.vector.tensor_tensor(out=mu[:], in0=stats[:, 0:2],
                                        in1=incols[:], op=mult)
                nc.vector.tensor_tensor(out=var[:], in0=stats[:, 2:4],
                                        in1=incols[:], op=mult)
                nc.vector.tensor_tensor(out=scl[:], in0=mu[:], in1=mu[:],
                                        op=mult)
                nc.vector.tensor_tensor(out=var[:], in0=var[:], in1=scl[:],
                                        op=sub)
                nc.vector.tensor_scalar_add(out=var[:], in0=var[:],
                                            scalar1=float(BN_EPS))
                nc.scalar.sqrt(out=var[:], in_=var[:])
                nc.vector.reciprocal(out=var[:], in_=var[:])
                nc.vector.tensor_tensor(out=scl[:], in0=var[:], in1=gcols[:],
                                        op=mult)
                nc.vector.tensor_tensor(out=bia[:], in0=mu[:], in1=scl[:],
                                        op=mult)
                nc.vector.tensor_tensor(out=bia[:], in0=becols[:], in1=bia[:],
                                        op=sub)

                # ---------- pass 2: BN+relu from h1, W2, output ----------
                rt = {}
                for s in range(S):
                    h = hA if s < 2 else hB
                    sl = slice(0, 64) if s % 2 == 0 else slice(64, 128)
                    r = rts.tile([64, SLOTS], bf16, tag=f"rt_{s}",
                                 name=f"rt_{s}")
                    nc.scalar.activation(
                        out=r[:], in_=h[sl, :], func=Relu,
                        bias=bia[:, 0:1], scale=scl[:, 0:1])
                    rt[s] = r
                rC = rts.tile([64, SLOTS], bf16, tag="rt_c", name="rt_c")
                nc.scalar.activation(
                    out=rC[:], in_=hC[:], func=Relu,
                    bias=bia[:, 1:2], scale=scl[:, 1:2])
                for gi, (q0, nq) in enumerate(chunks):
                    c0 = q0 * P
                    cw = nq * P
                    cn = min(PC - c0, cw)
                    pag = po.tile([64, 512], f32, tag="pag",
                                  name=f"pag_{gi}", space="PSUM")
                    nc.tensor.matmul(out=pag[:, 0:cn], lhsT=w2t[:, 64:128],
                                     rhs=rC[:, c0:c0 + cn],
                                     start=True, stop=True)
                    aggsb = osb.tile([64, 512], f32, tag="aggsb",
                                     name=f"aggsb_{gi}")
                    nc.vector.tensor_tensor(
                        out=aggsb[:, 0:cn], in0=pag[:, 0:cn],
                        in1=b2t[:, 0:1].to_broadcast([64, cn]), op=add)
                    for s in range(S):
                        pso = po.tile([64, 512], f32, tag="pso",
                                      name=f"pso_{gi}_{s}", space="PSUM")
                        nc.tensor.matmul(out=pso[:, 0:cn], lhsT=w2t[:, 0:64],
                                         rhs=rt[s][:, c0:c0 + cn],
                                         start=True, stop=True)
                        ot = osb.tile([64, 512], f32, tag="ot",
                                      name=f"ot_{gi}_{s}")
                        nc.vector.tensor_tensor(
                            out=ot[:, 0:cn], in0=pso[:, 0:cn],
                            in1=aggsb[:, 0:cn], op=add)
                        nc.sync.dma_start(
                            out=o_ap[s * 64:(s + 1) * 64, c0:c0 + cn],
                            in_=ot[:, 0:cn])
    # Align each gather/scatter's SWDGE queue with its post-schedule DMASW
    # sem lane (8 lanes round-robin over Pool DMA insts): lane % NQ.  The
    # scheduler reorders Pool DMA insts, so emission-order rotation would
    # put two queues on one sem lane (rejected by ucode ring metadata).
    import concourse.mybir as _mb
    idx = 0
    for blk in nc.m.functions[0].blocks:
        for inst in blk.instructions:
            tn = type(inst).__name__
            if tn in ("InstDMAGatherAnt", "InstDMAScatterAddAnt"):
                inst.queue_num = (idx % 8) % NQ
                idx += 1
            elif tn == "InstDMACopy" and inst.engine == _mb.EngineType.Pool:
                raise AssertionError("unexpected Pool-engine DMACopy")
    nc.compile()
    return nc


def kernel(**inputs):
    global last_exec_time_ns
    from concourse import bass_utils

    x = np.asarray(inputs["x"], np.float32)
    c = np.asarray(inputs["c"], np.float32)
    edge_index = np.asarray(inputs["edge_index"])
    eps_s = float(np.asarray(inputs["eps_shared"]))
    eps_a = float(np.asarray(inputs["eps_agg"]))
    args = [np.asarray(inputs[k], np.float32) for k in
            ("W1s", "g1s", "be1s", "W2s", "b2s",
             "W1a", "g1a", "be1a", "W2a", "b2a")]

    ret_maps, pre_maps, meta, orders = _build_inputs(
        x, c, edge_index, eps_a, *args)
    in_maps = _finish_inputs(ret_maps, pre_maps, eps_s, eps_a)
    clo = in_maps[0]["ilo"].shape[1] * 16
    chi = in_maps[0]["ihi"].shape[1] * 16
    key = (meta["lo"], meta["hi"], eps_s, eps_a, clo, chi)
    if key not in _prog_cache:
        _prog_cache[key] = _build_program(meta, eps_s, eps_a, clo, chi)
    nc = _prog_cache[key]

    kwargs = {}
    if _trace:
        try:
            import axon_profile_shim  # noqa: F401
        except ImportError:
            pass
        kwargs["trace"] = True
    res = bass_utils.run_bass_kernel_spmd(
        nc, in_maps, core_ids=list(range(NCORES)), **kwargs)
    last_exec_time_ns = res.exec_time_ns

    out = np.empty((N, S * EMB), dtype=np.float32)
    for k in range(NCORES):
        ok = res.results[k]["o"]          # [256, PC]
        out[k * PC + orders[k]] = ok.T
    return out


# revision 22
# speedup vs baseline: 1.2038x; 1.2038x over previous
"""ColourCatDSSGINConv on 8 trn2 NeuronCores.

Sharding: nodes are partitioned into 8 contiguous blocks of 6250; each core
aggregates the in-edges of its own nodes (pull model) from a replicated
node-feature table U = [x | c] (48 cols, padded to 64), then runs both GIN
MLP paths on its block.

Aggregation: per-phase (src<32768 / src>=32768, int16 gather indices) ELL
iterations over degree-sorted nodes; dense-prefix dma_gather calls (2048
rows, 256B each) round-robined over 4 SWDGE queues so descriptor
generation runs on all 8 Q7 cores concurrently; DVE adds accumulate into
SBUF.  The hi-phase accumulator is merged into canonical (lo) order with
dma_scatter_add into a DRAM table.

The epilogue is pipelined INTO the lo gather stream: as soon as the last
ELL iteration touching a 512-node chunk retires, that chunk's stacked
features are built, transposed, and pushed through the W1 matmul; BN
statistics (colsum + second-moment matmul) accumulate per chunk.  After
the last chunk: stats fold -> AllReduce[64,4] -> BN coefficients -> pass 2
(BN+relu from stored bf16 h1, W2 matmuls, output DMA).  b1s/b1a cancel
inside BatchNorm and are dropped.  Output is feature-major [256, 6250]
per core; the host transposes.
"""
import os
import sys

sys.path.insert(0, "/opt/trn_rl_repo")

import numpy as np

N = 50000
E = 800000
IN = 16
CD = 8
S = 4
EMB = 64
D = IN + CD          # 24
H = 64
BN_EPS = 1e-5

NCORES = 8
P = 128
PC = N // NCORES     # 6250 nodes per core
Q = (PC + P - 1) // P          # 49 column-groups of 128 nodes
SLOTS = Q * P                  # 6272
TROWS = 50176                  # 1 zero row + 50000 nodes + zero pad
LO_ROWS = 32768                # lo window rows [0, 32768): node v at row v+1
HI_BASE = 32768                # hi window: node v (>=32767) at row v+1
HI_ZERO = 50001 - HI_BASE      # a guaranteed-zero row in the hi window
LO_MAX_NODE = 32766
CALL = 1024                    # idxs per dma_gather call
SCALL = 1024                   # idxs per dma_scatter_add call
NQ = 4                         # swdge queues

last_exec_time_ns = None
_prog_cache = {}
_trace = bool(os.environ.get("GNN_TRACE"))


def _wrap16(flat, pad_val, pad_to):
    """int16 flat token list -> [128, pad_to//16] wrapped+replicated layout
    (token t lives at [t%16, t//16], replicated across the 8 gpsimd groups)."""
    n = len(flat)
    assert pad_to % 16 == 0 and n <= pad_to
    buf = np.full(pad_to, pad_val, dtype=np.int16)
    buf[:n] = flat
    arr = buf.reshape(pad_to // 16, 16).T.copy()
    return np.tile(arr, (8, 1))


def _phase_prep(src, ldst, lo):
    """ELL prep for one (core, phase): degree-desc node order; every edge gets
    (slot t, iteration j, int16 table idx)."""
    deg = np.bincount(ldst, minlength=PC)
    order = np.argsort(-deg, kind="stable")
    rank = np.empty(PC, dtype=np.int64)
    rank[order] = np.arange(PC)
    sidx = np.argsort(ldst, kind="stable")
    sd = ldst[sidx]
    ss = src[sidx]
    starts = np.searchsorted(sd, np.arange(PC))
    j = np.arange(len(sd)) - starts[sd]
    t = rank[sd]
    val = (ss + 1 - (0 if lo else HI_BASE)).astype(np.int16)
    return deg[order], order, t, j, val


def _pieces(n_pad, call):
    """Per gather-call DVE-add pieces: [(stg_j0, nj, acc_q0), ...] per call."""
    offs = [0]
    for n in n_pad:
        offs.append(offs[-1] + n)
    L = offs[-1]
    Lpad = ((L + call - 1) // call) * call
    out = []
    for c0 in range(0, Lpad, call):
        c1 = c0 + call
        ps = []
        for j, n in enumerate(n_pad):
            a, b = max(c0, offs[j]), min(c1, offs[j + 1])
            if a < b:
                ps.append(((a - c0) // P, (b - a) // P, (a - offs[j]) // P))
        out.append(ps)
    return out, Lpad


def _build_inputs(x, c, edge_index, eps_a, W1s, g1s, be1s, W2s, b2s,
                  W1a, g1a, be1a, W2a, b2a):
    src_all = edge_index[0].astype(np.int64)
    dst_all = edge_index[1].astype(np.int64)

    U = np.zeros((TROWS, 64), dtype=np.float32)
    U[1:N + 1, :IN] = x
    U[1:N + 1, IN:48] = c.reshape(N, S * CD)

    core_of = dst_all // PC
    meta = {}
    idx_arrays = {}
    scat_arrays = {}
    per = {}
    for k in range(NCORES):
        m = core_of == k
        s_k = src_all[m]
        d_k = dst_all[m] % PC
        lo_m = s_k <= LO_MAX_NODE
        per[(k, "lo")] = _phase_prep(s_k[lo_m], d_k[lo_m], True)
        per[(k, "hi")] = _phase_prep(s_k[~lo_m], d_k[~lo_m], False)

    for ph in ("lo", "hi"):
        maxdeg = max(int(per[(k, ph)][0][0]) if per[(k, ph)][0].size else 0
                     for k in range(NCORES))
        n_pad = []
        for j in range(maxdeg):
            nj = max(int(np.sum(per[(k, ph)][0] > j)) for k in range(NCORES))
            n_pad.append(((nj + P - 1) // P) * P)
        offs = np.concatenate([[0], np.cumsum(n_pad)]).astype(np.int64)
        _, Lpad = _pieces(n_pad, CALL)
        zero_idx = 0 if ph == "lo" else HI_ZERO
        meta[ph] = tuple(n_pad)
        for k in range(NCORES):
            _deg, order, t, j, val = per[(k, ph)]
            flat = np.full(Lpad, zero_idx, dtype=np.int16)
            flat[offs[j] + t] = val
            idx_arrays[(k, ph)] = _wrap16(flat, zero_idx, Lpad)
    # canonical per-core slot order = lo-phase order; only hi needs a merge
    orders = {}
    for k in range(NCORES):
        order_lo = per[(k, "lo")][1]
        rank_lo = np.empty(PC, dtype=np.int64)
        rank_lo[order_lo] = np.arange(PC)
        order_hi = per[(k, "hi")][1]
        ids = np.concatenate([rank_lo[order_hi],
                              np.arange(PC, SLOTS)]).astype(np.int16)
        scat_arrays[(k, "hi")] = _wrap16(ids, 0, SLOTS)
        orders[k] = order_lo

    import ml_dtypes
    wall = np.zeros((128, 320), dtype=np.float32)
    for s in range(S):
        wall[0:IN, s * H:(s + 1) * H] = W1s[0:IN, :]
        wall[IN + CD * s:IN + CD * (s + 1), s * H:(s + 1) * H] = W1s[IN:D, :]
    wall[64:64 + IN, 256:320] = W1a[0:IN, :]
    wall[64 + IN:64 + D, 256:320] = W1a[IN:D, :]
    w2 = np.concatenate([W2s, W2a], axis=1).astype(ml_dtypes.bfloat16)

    bnc = np.zeros((64, 6), dtype=np.float32)
    bnc[:, 0] = g1s
    bnc[:, 1] = be1s
    bnc[:, 2] = 1.0 / (N * S)
    bnc[:, 3] = g1a
    bnc[:, 4] = be1a
    bnc[:, 5] = 1.0 / N
    b2v = (b2s + b2a).astype(np.float32).reshape(64, 1)
    wallT = np.zeros((64, 640), dtype=np.float32)
    for b in range(5):
        wallT[:, b * 128:(b + 1) * 128] = wall[:, b * 64:(b + 1) * 64].T

    in_maps = []
    for k in range(NCORES):
        # own-side contribution to the stacked features, node-major
        uo = np.zeros((SLOTS, 128), dtype=np.float32)
        rows = U[1 + k * PC + orders[k]]
        uo[:PC, 0:48] = (1.0 + 0.0) * rows[:, 0:48]          # scaled below
        in_maps.append({"rows": rows, "uo": uo})

    ret_maps = []
    for k in range(NCORES):
        ret_maps.append({
            "utab": U,
            "ilo": idx_arrays[(k, "lo")],
            "ihi": idx_arrays[(k, "hi")],
            "shi": scat_arrays[(k, "hi")],
            "wall": wall.astype(ml_dtypes.bfloat16),
            "w2": w2,
            "bnc": bnc,
            "b2v": b2v,
            "wallf": wall,
            "wallT": wallT,
        })
    return ret_maps, in_maps, meta, orders


def _finish_inputs(ret_maps, pre_maps, eps_s, eps_a):
    for k in range(NCORES):
        rows = pre_maps[k]["rows"]
        uo = pre_maps[k]["uo"]
        uo[:PC, 0:48] = (1.0 + eps_s) * rows[:, 0:48]
        uo[:PC, 64:80] = (1.0 + eps_a) * rows[:, 0:IN]
        cmean = rows[:, IN:48].reshape(PC, S, CD).mean(axis=1)
        uo[:PC, 80:88] = (1.0 + eps_a) * cmean
        uo_t = uo.reshape(Q, P, 128).transpose(1, 0, 2).reshape(
            P, Q * 128).copy()
        ret_maps[k]["uown2"] = uo_t
    return ret_maps


def _build_program(meta, eps_s, eps_a, clo, chi):
    import concourse.bacc as bacc
    import concourse.tile as tile
    import concourse.mybir as mybir
    from concourse.masks import make_identity

    f32 = mybir.dt.float32
    bf16 = mybir.dt.bfloat16
    i16 = mybir.dt.int16
    add = mybir.AluOpType.add
    sub = mybir.AluOpType.subtract
    mult = mybir.AluOpType.mult
    Relu = mybir.ActivationFunctionType.Relu

    nc = bacc.Bacc("TRN2", target_bir_lowering=False, debug=False,
                   num_devices=NCORES, num_swdge_queues=NQ)
    utab = nc.dram_tensor("utab", [TROWS, 64], f32, kind="ExternalInput").ap()
    uown2 = nc.dram_tensor("uown2", [P, Q * 128], f32,
                           kind="ExternalInput").ap()
    ilo = nc.dram_tensor("ilo", [128, clo // 16], i16, kind="ExternalInput").ap()
    ihi = nc.dram_tensor("ihi", [128, chi // 16], i16, kind="ExternalInput").ap()
    shi = nc.dram_tensor("shi", [128, SLOTS // 16], i16, kind="ExternalInput").ap()
    wallp = nc.dram_tensor("wall", [128, 320], bf16, kind="ExternalInput").ap()
    w2p = nc.dram_tensor("w2", [64, 128], bf16, kind="ExternalInput").ap()
    bncp = nc.dram_tensor("bnc", [64, 6], f32, kind="ExternalInput").ap()
    b2vp = nc.dram_tensor("b2v", [64, 1], f32, kind="ExternalInput").ap()
    wallfp = nc.dram_tensor("wallf", [128, 320], f32, kind="ExternalInput").ap()
    wallTp = nc.dram_tensor("wallT", [64, 640], f32, kind="ExternalInput").ap()
    o_ap = nc.dram_tensor("o", [256, PC], f32, kind="ExternalOutput").ap()

    # chunks of up to 4 column-groups (512 slots)
    chunks = [(q0, min(4, Q - q0)) for q0 in range(0, Q, 4)]
    pieces_lo, _ = _pieces(list(meta["lo"]), CALL)
    pieces_hi, _ = _pieces(list(meta["hi"]), CALL)
    last_call_q = [0] * Q
    for ci, ps in enumerate(pieces_lo):
        for (_sj, nj, qa) in ps:
            for q in range(qa, min(qa + nj, Q)):
                last_call_q[q] = ci
    ready_at = {}
    for gi, (q0, nq) in enumerate(chunks):
        rc = max(last_call_q[q0:q0 + nq])
        ready_at.setdefault(rc, []).append(gi)

    with tile.TileContext(nc) as tc:
        with (
            tc.tile_pool(name="dram", bufs=1, space="DRAM") as dp,
            tc.tile_pool(name="persist", bufs=1) as mp,
        ):
            accd = dp.tile([SLOTS, 64], f32)

            # ---------- persistent tiles (idx tables first: gathers wait) ----
            it_hi = mp.tile([128, chi // 16], i16, tag="it_hi")
            nc.sync.dma_start(out=it_hi[:], in_=ihi[:])
            it_lo = mp.tile([128, clo // 16], i16, tag="it_lo")
            nc.sync.dma_start(out=it_lo[:], in_=ilo[:])
            st = mp.tile([128, SLOTS // 16], i16, tag="st_hi")
            nc.sync.dma_start(out=st[:], in_=shi[:])
            wt = mp.tile([P, 320], bf16, tag="wt")
            nc.sync.dma_start(out=wt[:], in_=wallp[:])
            wtf = mp.tile([P, 320], f32, tag="wtf")
            nc.sync.dma_start(out=wtf[:], in_=wallfp[:])
            wtT = mp.tile([64, 640], f32, tag="wtT")
            nc.sync.dma_start(out=wtT[:], in_=wallTp[:])
            w2t = mp.tile([64, 128], bf16, tag="w2t")
            nc.sync.dma_start(out=w2t[:], in_=w2p[:])
            bnct = mp.tile([64, 6], f32, tag="bnct")
            nc.sync.dma_start(out=bnct[:], in_=bncp[:])
            b2t = mp.tile([64, 1], f32, tag="b2t")
            nc.sync.dma_start(out=b2t[:], in_=b2vp[:])
            ident = mp.tile([P, P], bf16, tag="ident")
            make_identity(nc, ident[:])
            own2 = mp.tile([P, Q, 128], f32, tag="own2")
            nc.sync.dma_start(out=own2[:].rearrange("p q d -> p (q d)"),
                              in_=uown2[:])

            acc_lo = mp.tile([P, Q, 64], f32, tag="acc_lo")
            nc.vector.memset(acc_lo[:].rearrange("p q d -> p (q d)"), 0.0)
            acc_hi = mp.tile([P, Q, 64], f32, tag="acc_hi")
            nc.vector.memset(acc_hi[:].rearrange("p q d -> p (q d)"), 0.0)
            accm = mp.tile([P, Q, 64], f32, tag="accm")
            csum = mp.tile([P, 1], f32, tag="csum")
            nc.vector.memset(csum[:], 0.0)
            hA = mp.tile([P, SLOTS], bf16, tag="hA")
            hB = mp.tile([P, SLOTS], bf16, tag="hB")
            hC = mp.tile([64, SLOTS], bf16, tag="hC")
            stats = mp.tile([64, 4], f32, tag="stats")
            m2s = mp.tile([P, P], f32, tag="m2s")

            qrot = [0]

            def rr():
                qn = qrot[0] % NQ
                qrot[0] += 1
                return qn

            # ---------- phase 1: gather + ELL accumulate + chunk epilogue ---
            with (
                tc.tile_pool(name="stg", bufs=12) as sp,
                tc.tile_pool(name="zp", bufs=1) as zp,
                tc.tile_pool(name="stkp", bufs=3) as stkp,
                tc.tile_pool(name="ptp", bufs=1, space="PSUM") as ptp,
                tc.tile_pool(name="pw1", bufs=2, space="PSUM") as pw1,
                tc.tile_pool(name="pm2m", bufs=1, space="PSUM") as pm2m,
            ):
                zt = zp.tile([P, Q * 64], f32, tag="zt")
                nc.vector.memset(zt[:], 0.0)
                nc.sync.dma_start(
                    out=accd[:].rearrange("(q p) d -> p q d", p=P),
                    in_=zt[:].rearrange("p (q d) -> p q d", d=64))

                m2 = pm2m.tile([P, P], f32, tag="m2", space="PSUM")
                m2_cnt = [0]

                def emit_chunk(gi):
                    q0, nq = chunks[gi]
                    cw = nq * P
                    c0 = q0 * P
                    stk = stkp.tile([P, 4, 128], f32, tag="stk",
                                    name=f"stk_{gi}")
                    nc.sync.dma_start(
                        out=stk[:, 0:nq, :].rearrange("p q d -> p (q d)"),
                        in_=uown2[:, q0 * 128:(q0 + nq) * 128])
                    # shared msg cols 0:48 (u-agg cols 0:48); wall reads 0:64
                    nc.vector.tensor_tensor(
                        out=stk[:, 0:nq, 0:48], in0=stk[:, 0:nq, 0:48],
                        in1=accm[:, q0:q0 + nq, 0:48], op=add)
                    nc.vector.tensor_tensor(
                        out=stk[:, 0:nq, 0:48], in0=stk[:, 0:nq, 0:48],
                        in1=acc_lo[:, q0:q0 + nq, 0:48], op=add)
                    # agg-path x part cols 64:80
                    nc.vector.tensor_tensor(
                        out=stk[:, 0:nq, 64:80], in0=stk[:, 0:nq, 64:80],
                        in1=accm[:, q0:q0 + nq, 0:16], op=add)
                    nc.vector.tensor_tensor(
                        out=stk[:, 0:nq, 64:80], in0=stk[:, 0:nq, 64:80],
                        in1=acc_lo[:, q0:q0 + nq, 0:16], op=add)
                    # agg-path c part cols 80:88: own + 0.25*sum_s agg_c_s
                    tcc = stkp.tile([P, 4, 32], f32, tag="tcc",
                                    name=f"tcc_{gi}")
                    nc.vector.tensor_tensor(
                        out=tcc[:, 0:nq, :], in0=accm[:, q0:q0 + nq, 16:48],
                        in1=acc_lo[:, q0:q0 + nq, 16:48], op=add)
                    nc.vector.tensor_tensor(
                        out=tcc[:, 0:nq, 0:8], in0=tcc[:, 0:nq, 0:8],
                        in1=tcc[:, 0:nq, 8:16], op=add)
                    nc.vector.tensor_tensor(
                        out=tcc[:, 0:nq, 16:24], in0=tcc[:, 0:nq, 16:24],
                        in1=tcc[:, 0:nq, 24:32], op=add)
                    nc.vector.tensor_tensor(
                        out=tcc[:, 0:nq, 0:8], in0=tcc[:, 0:nq, 0:8],
                        in1=tcc[:, 0:nq, 16:24], op=add)
                    nc.vector.scalar_tensor_tensor(
                        out=stk[:, 0:nq, 80:88], in0=tcc[:, 0:nq, 0:8],
                        scalar=0.25, op0=mult,
                        in1=stk[:, 0:nq, 80:88], op1=add)
                    # bf16 + transpose to feature-major
                    stkb = stkp.tile([P, 4, 128], bf16, tag="stkb",
                                     name=f"stkb_{gi}")
                    nc.scalar.copy(
                        out=stkb[:, 0:nq, :].rearrange("p q d -> p (q d)"),
                        in_=stk[:, 0:nq, :].rearrange("p q d -> p (q d)"))
                    tp = ptp.tile([P, 4, P], bf16, tag="tp", name=f"tp_{gi}",
                                  space="PSUM")
                    for qq in range(nq):
                        nc.tensor.transpose(out=tp[:, qq, :],
                                            in_=stkb[:, qq, :],
                                            identity=ident[:])
                        nc.tensor.matmul(out=m2[:], lhsT=stkb[:, qq, :],
                                         rhs=stkb[:, qq, :],
                                         start=(m2_cnt[0] == 0),
                                         stop=(m2_cnt[0] == Q - 1))
                        m2_cnt[0] += 1
                    stT = stkp.tile([P, 512], bf16, tag="stT",
                                    name=f"stT_{gi}")
                    nc.scalar.copy(
                        out=stT[:, 0:cw],
                        in_=tp[:, 0:nq, :].rearrange("p q d -> p (q d)"))
                    # colsum accumulate (over stacked features)
                    redS = stkp.tile([P, 1], f32, tag="redS",
                                     name=f"redS_{gi}")
                    nc.vector.tensor_reduce(out=redS[:], in_=stT[:, 0:cw],
                                            axis=mybir.AxisListType.X, op=add)
                    nc.vector.tensor_tensor(out=csum[:], in0=csum[:],
                                            in1=redS[:], op=add)
                    # W1 matmuls -> h1 (bf16, no BN yet)
                    psA = pw1.tile([P, 512], f32, tag="psA",
                                   name=f"psA_{gi}", space="PSUM")
                    nc.tensor.matmul(out=psA[:, 0:cw], lhsT=wt[:, 0:128],
                                     rhs=stT[:, 0:cw], start=True, stop=True)
                    nc.scalar.copy(out=hA[:, c0:c0 + cw], in_=psA[:, 0:cw])
                    psB = pw1.tile([P, 512], f32, tag="psB",
                                   name=f"psB_{gi}", space="PSUM")
                    nc.tensor.matmul(out=psB[:, 0:cw], lhsT=wt[:, 128:256],
                                     rhs=stT[:, 0:cw], start=True, stop=True)
                    nc.scalar.copy(out=hB[:, c0:c0 + cw], in_=psB[:, 0:cw])
                    psC = pw1.tile([64, 512], f32, tag="psC",
                                   name=f"psC_{gi}", space="PSUM")
                    nc.tensor.matmul(out=psC[:, 0:cw], lhsT=wt[:, 256:320],
                                     rhs=stT[:, 0:cw], start=True, stop=True)
                    nc.scalar.copy(out=hC[:, c0:c0 + cw], in_=psC[:, 0:cw])

                # --- hi phase: gathers + adds, then scatter-merge ---
                for ci, ps in enumerate(pieces_hi):
                    g = sp.tile([P, CALL // P, 64], f32, tag="stg",
                                name=f"g_hi_{ci}")
                    nc.gpsimd.dma_gather(
                        out_ap=g[:], in_ap=utab[HI_BASE:TROWS, :],
                        idxs_ap=it_hi[:, ci * (CALL // 16):
                                      (ci + 1) * (CALL // 16)],
                        num_idxs=CALL, num_idxs_reg=CALL, elem_size=64,
                        queue_num=rr())
                    for (sj, nj, qa) in ps:
                        nc.vector.tensor_tensor(
                            out=acc_hi[:, qa:qa + nj, :],
                            in0=acc_hi[:, qa:qa + nj, :],
                            in1=g[:, sj:sj + nj, :], op=add)
                for si, c0 in enumerate(range(0, SLOTS, SCALL)):
                    n = min(SCALL, SLOTS - c0)
                    nc.gpsimd.dma_scatter_add(
                        accd[:], acc_hi[:, c0 // P:(c0 + n) // P, :],
                        st[:, c0 // 16:(c0 + n) // 16],
                        num_idxs=n, num_idxs_reg=n, elem_size=64,
                        queue_num=rr())
                nc.sync.dma_start(
                    out=accm[:],
                    in_=accd[:].rearrange("(q p) d -> p q d", p=P))

                # --- lo phase with interleaved chunk work ---
                for ci, ps in enumerate(pieces_lo):
                    g = sp.tile([P, CALL // P, 64], f32, tag="stg",
                                name=f"g_lo_{ci}")
                    nc.gpsimd.dma_gather(
                        out_ap=g[:], in_ap=utab[0:LO_ROWS, :],
                        idxs_ap=it_lo[:, ci * (CALL // 16):
                                      (ci + 1) * (CALL // 16)],
                        num_idxs=CALL, num_idxs_reg=CALL, elem_size=64,
                        queue_num=rr())
                    for (sj, nj, qa) in ps:
                        nc.vector.tensor_tensor(
                            out=acc_lo[:, qa:qa + nj, :],
                            in0=acc_lo[:, qa:qa + nj, :],
                            in1=g[:, sj:sj + nj, :], op=add)
                    for gi in ready_at.get(ci, []):
                        emit_chunk(gi)
                nc.vector.tensor_copy(out=m2s[:], in_=m2[:])

            # ---------- stats fold + AllReduce + BN + pass 2 ----------
            cin = dp.tile([64, 4], f32)
            cout = dp.tile([64, 4], f32)
            with (
                tc.tile_pool(name="po", bufs=2, space="PSUM") as po,
                tc.tile_pool(name="pagp", bufs=1, space="PSUM") as pagp,
                tc.tile_pool(name="pst", bufs=1, space="PSUM") as pst,
                tc.tile_pool(name="rts", bufs=1) as rts,
                tc.tile_pool(name="outs", bufs=3) as osb,
                tc.tile_pool(name="bnp", bufs=1) as bnp,
            ):
                # s1 (sum of z) per 64-block via wall^T @ colsum;
                # s2 (sum of z^2) via w^T M2 w per block
                s1 = bnp.tile([64, 2], f32, tag="s1")
                s2 = bnp.tile([64, 2], f32, tag="s2")
                nc.vector.memset(s1[:], 0.0)
                nc.vector.memset(s2[:], 0.0)
                for b in range(5):
                    col = 0 if b < 4 else 1
                    wm = pst.tile([64, P], f32, tag="wm", name=f"wm_{b}",
                                  space="PSUM")
                    nc.tensor.matmul(out=wm[:], lhsT=wtf[:, b * 64:(b + 1) * 64],
                                     rhs=m2s[:], start=True, stop=True)
                    prod = bnp.tile([64, P], f32, tag="prod", name=f"prod_{b}")
                    nc.vector.tensor_tensor(
                        out=prod[:], in0=wm[:],
                        in1=wtT[:, b * 128:(b + 1) * 128], op=mult)
                    r2 = bnp.tile([64, 1], f32, tag="r2", name=f"r2_{b}")
                    nc.vector.tensor_reduce(out=r2[:], in_=prod[:],
                                            axis=mybir.AxisListType.X, op=add)
                    nc.vector.tensor_tensor(
                        out=s2[:, col:col + 1], in0=s2[:, col:col + 1],
                        in1=r2[:], op=add)
                    p1 = pst.tile([64, 1], f32, tag="p1", name=f"p1_{b}",
                                  space="PSUM")
                    nc.tensor.matmul(out=p1[:], lhsT=wtf[:, b * 64:(b + 1) * 64],
                                     rhs=csum[:], start=True, stop=True)
                    r1 = bnp.tile([64, 1], f32, tag="r1", name=f"r1_{b}")
                    nc.vector.tensor_copy(out=r1[:], in_=p1[:])
                    nc.vector.tensor_tensor(
                        out=s1[:, col:col + 1], in0=s1[:, col:col + 1],
                        in1=r1[:], op=add)
                nc.vector.tensor_copy(out=stats[:, 0:2], in_=s1[:])
                nc.vector.tensor_copy(out=stats[:, 2:4], in_=s2[:])

                nc.sync.dma_start(out=cin[:], in_=stats[:])
                nc.gpsimd.collective_compute(
                    "AllReduce", add,
                    replica_groups=[list(range(NCORES))],
                    ins=[cin.opt()], outs=[cout.opt()])
                nc.sync.dma_start(out=stats[:], in_=cout[:])

                gcols = bnp.tile([64, 2], f32, tag="gcols")
                becols = bnp.tile([64, 2], f32, tag="becols")
                incols = bnp.tile([64, 2], f32, tag="incols")
                nc.vector.tensor_copy(out=gcols[:, 0:1], in_=bnct[:, 0:1])
                nc.vector.tensor_copy(out=gcols[:, 1:2], in_=bnct[:, 3:4])
                nc.vector.tensor_copy(out=becols[:, 0:1], in_=bnct[:, 1:2])
                nc.vector.tensor_copy(out=becols[:, 1:2], in_=bnct[:, 4:5])
                nc.vector.tensor_copy(out=incols[:, 0:1], in_=bnct[:, 2:3])
                nc.vector.tensor_copy(out=incols[:, 1:2], in_=bnct[:, 5:6])
                mu = bnp.tile([64, 2], f32, tag="mu")
                var = bnp.tile([64, 2], f32, tag="var")
                scl = bnp.tile([64, 2], f32, tag="scl")
                bia = bnp.tile([64, 2], f32, tag="bia")
                nc.vector.tensor_tensor(out=mu[:], in0=stats[:, 0:2],
                                        in1=incols[:], op=mult)
                nc.vector.tensor_tensor(out=var[:], in0=stats[:, 2:4],
                                        in1=incols[:], op=mult)
                nc.vector.tensor_tensor(out=scl[:], in0=mu[:], in1=mu[:],
                                        op=mult)
                nc.vector.tensor_tensor(out=var[:], in0=var[:], in1=scl[:],
                                        op=sub)
                nc.vector.tensor_scalar_add(out=var[:], in0=var[:],
                                            scalar1=float(BN_EPS))
                nc.scalar.sqrt(out=var[:], in_=var[:])
                nc.vector.reciprocal(out=var[:], in_=var[:])
                nc.vector.tensor_tensor(out=scl[:], in0=var[:], in1=gcols[:],
                                        op=mult)
                nc.vector.tensor_tensor(out=bia[:], in0=mu[:], in1=scl[:],
                                        op=mult)
                nc.vector.tensor_tensor(out=bia[:], in0=becols[:], in1=bia[:],
                                        op=sub)

                # ---------- pass 2: BN+relu from h1, W2, output ----------
                rt = {}
                for s in range(S):
                    h = hA if s < 2 else hB
                    sl = slice(0, 64) if s % 2 == 0 else slice(64, 128)
                    r = rts.tile([64, SLOTS], bf16, tag=f"rt_{s}",
                                 name=f"rt_{s}")
                    nc.scalar.activation(
                        out=r[:], in_=h[sl, :], func=Relu,
                        bias=bia[:, 0:1], scale=scl[:, 0:1])
                    rt[s] = r
                rC = rts.tile([64, SLOTS], bf16, tag="rt_c", name="rt_c")
                nc.scalar.activation(
                    out=rC[:], in_=hC[:], func=Relu,
                    bias=bia[:, 1:2], scale=scl[:, 1:2])
                PW = 512
                for gi, c0 in enumerate(range(0, PC, PW)):
                    cn = min(PC - c0, PW)
                    pag = pagp.tile([64, PW], f32, tag="pag",
                                    name=f"pag_{gi}", space="PSUM")
                    nc.tensor.matmul(out=pag[:, 0:cn], lhsT=w2t[:, 64:128],
                                     rhs=rC[:, c0:c0 + cn],
                                     start=True, stop=True)
                    aggsb = osb.tile([64, PW], f32, tag="aggsb",
                                     name=f"aggsb_{gi}")
                    nc.vector.tensor_tensor(
                        out=aggsb[:, 0:cn], in0=pag[:, 0:cn],
                        in1=b2t[:, 0:1].to_broadcast([64, cn]), op=add)
                    for s in range(S):
                        pso = po.tile([64, PW], f32, tag="pso",
                                      name=f"pso_{gi}_{s}", space="PSUM")
                        nc.tensor.matmul(out=pso[:, 0:cn], lhsT=w2t[:, 0:64],
                                         rhs=rt[s][:, c0:c0 + cn],
                                         start=True, stop=True)
                        ot = osb.tile([64, PW], f32, tag="ot",
                                      name=f"ot_{gi}_{s}")
                        nc.vector.tensor_tensor(
                            out=ot[:, 0:cn], in0=pso[:, 0:cn],
                            in1=aggsb[:, 0:cn], op=add)
                        nc.sync.dma_start(
                            out=o_ap[s * 64:(s + 1) * 64, c0:c0 + cn],
                            in_=ot[:, 0:cn])
    # Align each gather/scatter's SWDGE queue with its post-schedule DMASW
    # sem lane (8 lanes round-robin over Pool DMA insts): lane % NQ.  The
    # scheduler reorders Pool DMA insts, so emission-order rotation would
    # put two queues on one sem lane (rejected by ucode ring metadata).
    import concourse.mybir as _mb
    idx = 0
    for blk in nc.m.functions[0].blocks:
        for inst in blk.instructions:
            tn = type(inst).__name__
            if tn in ("InstDMAGatherAnt", "InstDMAScatterAddAnt"):
                inst.queue_num = (idx % 8) % NQ
                idx += 1
            elif tn == "InstDMACopy" and inst.engine == _mb.EngineType.Pool:
                raise AssertionError("unexpected Pool-engine DMACopy")
    nc.compile()
    return nc


def kernel(**inputs):
    global last_exec_time_ns
    from concourse import bass_utils

    x = np.asarray(inputs["x"], np.float32)
    c = np.asarray(inputs["c"], np.float32)
    edge_index = np.asarray(inputs["edge_index"])
    eps_s = float(np.asarray(inputs["eps_shared"]))
    eps_a = float(np.asarray(inputs["eps_agg"]))
    args = [np.asarray(inputs[k], np.float32) for k in
            ("W1s", "g1s", "be1s", "W2s", "b2s",
             "W1a", "g1a", "be1a", "W2a", "b2a")]

    ret_maps, pre_maps, meta, orders = _build_inputs(
        x, c, edge_index, eps_a, *args)
    in_maps = _finish_inputs(ret_maps, pre_maps, eps_s, eps_a)
    clo = in_maps[0]["ilo"].shape[1] * 16
    chi = in_maps[0]["ihi"].shape[1] * 16
    key = (meta["lo"], meta["hi"], eps_s, eps_a, clo, chi)
    if key not in _prog_cache:
        _prog_cache[key] = _build_program(meta, eps_s, eps_a, clo, chi)
    nc = _prog_cache[key]

    kwargs = {}
    if _trace:
        try:
            import axon_profile_shim  # noqa: F401
        except ImportError:
            pass
        kwargs["trace"] = True
    res = bass_utils.run_bass_kernel_spmd(
        nc, in_maps, core_ids=list(range(NCORES)), **kwargs)
    last_exec_time_ns = res.exec_time_ns

    out = np.empty((N, S * EMB), dtype=np.float32)
    for k in range(NCORES):
        ok = res.results[k]["o"]          # [256, PC]
        out[k * PC + orders[k]] = ok.T
    return out


# revision 23
# speedup vs baseline: 1.2525x; 1.0404x over previous
"""ColourCatDSSGINConv on 8 trn2 NeuronCores.

Sharding: nodes are partitioned into 8 contiguous blocks of 6250; each core
aggregates the in-edges of its own nodes (pull model) from a replicated
node-feature table U = [x | c] (48 cols, padded to 64), then runs both GIN
MLP paths on its block.

Aggregation: per-phase (src<32768 / src>=32768, int16 gather indices) ELL
iterations over degree-sorted nodes; dense-prefix dma_gather calls (2048
rows, 256B each) round-robined over 4 SWDGE queues so descriptor
generation runs on all 8 Q7 cores concurrently; DVE adds accumulate into
SBUF.  The hi-phase accumulator is merged into canonical (lo) order with
dma_scatter_add into a DRAM table.

The epilogue is pipelined INTO the lo gather stream: as soon as the last
ELL iteration touching a 512-node chunk retires, that chunk's stacked
features are built, transposed, and pushed through the W1 matmul; BN
statistics (colsum + second-moment matmul) accumulate per chunk.  After
the last chunk: stats fold -> AllReduce[64,4] -> BN coefficients -> pass 2
(BN+relu from stored bf16 h1, W2 matmuls, output DMA).  b1s/b1a cancel
inside BatchNorm and are dropped.  Output is feature-major [256, 6250]
per core; the host transposes.
"""
import os
import sys

sys.path.insert(0, "/opt/trn_rl_repo")

import numpy as np

N = 50000
E = 800000
IN = 16
CD = 8
S = 4
EMB = 64
D = IN + CD          # 24
H = 64
BN_EPS = 1e-5

NCORES = 8
P = 128
PC = N // NCORES     # 6250 nodes per core
Q = (PC + P - 1) // P          # 49 column-groups of 128 nodes
SLOTS = Q * P                  # 6272
TROWS = 50176                  # 1 zero row + 50000 nodes + zero pad
LO_ROWS = 32768                # lo window rows [0, 32768): node v at row v+1
HI_BASE = 32768                # hi window: node v (>=32767) at row v+1
HI_ZERO = 50001 - HI_BASE      # a guaranteed-zero row in the hi window
LO_MAX_NODE = 32766
CALL = 1024                    # idxs per dma_gather call
SCALL = 1024                   # idxs per dma_scatter_add call
NQ = 4                         # swdge queues

last_exec_time_ns = None
_prog_cache = {}
_trace = bool(os.environ.get("GNN_TRACE"))


def _wrap16(flat, pad_val, pad_to):
    """int16 flat token list -> [128, pad_to//16] wrapped+replicated layout
    (token t lives at [t%16, t//16], replicated across the 8 gpsimd groups)."""
    n = len(flat)
    assert pad_to % 16 == 0 and n <= pad_to
    buf = np.full(pad_to, pad_val, dtype=np.int16)
    buf[:n] = flat
    arr = buf.reshape(pad_to // 16, 16).T.copy()
    return np.tile(arr, (8, 1))


def _phase_prep(src, ldst, lo):
    """ELL prep for one (core, phase): degree-desc node order; every edge gets
    (slot t, iteration j, int16 table idx)."""
    deg = np.bincount(ldst, minlength=PC)
    order = np.argsort(-deg, kind="stable")
    rank = np.empty(PC, dtype=np.int64)
    rank[order] = np.arange(PC)
    sidx = np.argsort(ldst, kind="stable")
    sd = ldst[sidx]
    ss = src[sidx]
    starts = np.searchsorted(sd, np.arange(PC))
    j = np.arange(len(sd)) - starts[sd]
    t = rank[sd]
    val = (ss + 1 - (0 if lo else HI_BASE)).astype(np.int16)
    return deg[order], order, t, j, val


def _pieces(n_pad, call):
    """Per gather-call DVE-add pieces: [(stg_j0, nj, acc_q0), ...] per call."""
    offs = [0]
    for n in n_pad:
        offs.append(offs[-1] + n)
    L = offs[-1]
    Lpad = ((L + call - 1) // call) * call
    out = []
    for c0 in range(0, Lpad, call):
        c1 = c0 + call
        ps = []
        for j, n in enumerate(n_pad):
            a, b = max(c0, offs[j]), min(c1, offs[j + 1])
            if a < b:
                ps.append(((a - c0) // P, (b - a) // P, (a - offs[j]) // P))
        out.append(ps)
    return out, Lpad


def _build_inputs(x, c, edge_index, eps_a, W1s, g1s, be1s, W2s, b2s,
                  W1a, g1a, be1a, W2a, b2a):
    src_all = edge_index[0].astype(np.int64)
    dst_all = edge_index[1].astype(np.int64)

    U = np.zeros((TROWS, 64), dtype=np.float32)
    U[1:N + 1, :IN] = x
    U[1:N + 1, IN:48] = c.reshape(N, S * CD)

    core_of = dst_all // PC
    meta = {}
    idx_arrays = {}
    scat_arrays = {}
    per = {}
    for k in range(NCORES):
        m = core_of == k
        s_k = src_all[m]
        d_k = dst_all[m] % PC
        lo_m = s_k <= LO_MAX_NODE
        per[(k, "lo")] = _phase_prep(s_k[lo_m], d_k[lo_m], True)
        per[(k, "hi")] = _phase_prep(s_k[~lo_m], d_k[~lo_m], False)

    for ph in ("lo", "hi"):
        maxdeg = max(int(per[(k, ph)][0][0]) if per[(k, ph)][0].size else 0
                     for k in range(NCORES))
        n_pad = []
        for j in range(maxdeg):
            nj = max(int(np.sum(per[(k, ph)][0] > j)) for k in range(NCORES))
            n_pad.append(((nj + P - 1) // P) * P)
        offs = np.concatenate([[0], np.cumsum(n_pad)]).astype(np.int64)
        _, Lpad = _pieces(n_pad, CALL)
        zero_idx = 0 if ph == "lo" else HI_ZERO
        meta[ph] = tuple(n_pad)
        for k in range(NCORES):
            _deg, order, t, j, val = per[(k, ph)]
            flat = np.full(Lpad, zero_idx, dtype=np.int16)
            flat[offs[j] + t] = val
            idx_arrays[(k, ph)] = _wrap16(flat, zero_idx, Lpad)
    # canonical per-core slot order = lo-phase order; only hi needs a merge
    orders = {}
    for k in range(NCORES):
        order_lo = per[(k, "lo")][1]
        rank_lo = np.empty(PC, dtype=np.int64)
        rank_lo[order_lo] = np.arange(PC)
        order_hi = per[(k, "hi")][1]
        ids = np.concatenate([rank_lo[order_hi],
                              np.arange(PC, SLOTS)]).astype(np.int16)
        scat_arrays[(k, "hi")] = _wrap16(ids, 0, SLOTS)
        orders[k] = order_lo

    import ml_dtypes
    wall = np.zeros((128, 320), dtype=np.float32)
    for s in range(S):
        wall[0:IN, s * H:(s + 1) * H] = W1s[0:IN, :]
        wall[IN + CD * s:IN + CD * (s + 1), s * H:(s + 1) * H] = W1s[IN:D, :]
    wall[64:64 + IN, 256:320] = W1a[0:IN, :]
    wall[64 + IN:64 + D, 256:320] = W1a[IN:D, :]
    w2 = np.concatenate([W2s, W2a], axis=1).astype(ml_dtypes.bfloat16)

    bnc = np.zeros((64, 6), dtype=np.float32)
    bnc[:, 0] = g1s
    bnc[:, 1] = be1s
    bnc[:, 2] = 1.0 / (N * S)
    bnc[:, 3] = g1a
    bnc[:, 4] = be1a
    bnc[:, 5] = 1.0 / N
    b2v = (b2s + b2a).astype(np.float32).reshape(64, 1)
    wallT = np.zeros((64, 640), dtype=np.float32)
    for b in range(5):
        wallT[:, b * 128:(b + 1) * 128] = wall[:, b * 64:(b + 1) * 64].T

    in_maps = []
    for k in range(NCORES):
        # own-side contribution to the stacked features, node-major
        uo = np.zeros((SLOTS, 128), dtype=np.float32)
        rows = U[1 + k * PC + orders[k]]
        uo[:PC, 0:48] = (1.0 + 0.0) * rows[:, 0:48]          # scaled below
        in_maps.append({"rows": rows, "uo": uo})

    ret_maps = []
    for k in range(NCORES):
        ret_maps.append({
            "utab": U,
            "ilo": idx_arrays[(k, "lo")],
            "ihi": idx_arrays[(k, "hi")],
            "shi": scat_arrays[(k, "hi")],
            "wall": wall.astype(ml_dtypes.bfloat16),
            "w2": w2,
            "bnc": bnc,
            "b2v": b2v,
            "wallf": wall,
            "wallT": wallT,
        })
    return ret_maps, in_maps, meta, orders


def _finish_inputs(ret_maps, pre_maps, eps_s, eps_a):
    for k in range(NCORES):
        rows = pre_maps[k]["rows"]
        uo = pre_maps[k]["uo"]
        uo[:PC, 0:48] = (1.0 + eps_s) * rows[:, 0:48]
        uo[:PC, 64:80] = (1.0 + eps_a) * rows[:, 0:IN]
        cmean = rows[:, IN:48].reshape(PC, S, CD).mean(axis=1)
        uo[:PC, 80:88] = (1.0 + eps_a) * cmean
        uo_t = uo.reshape(Q, P, 128).transpose(1, 0, 2).reshape(
            P, Q * 128).copy()
        ret_maps[k]["uown2"] = uo_t
    return ret_maps


def _build_program(meta, eps_s, eps_a, clo, chi):
    import concourse.bacc as bacc
    import concourse.tile as tile
    import concourse.mybir as mybir
    from concourse.masks import make_identity

    f32 = mybir.dt.float32
    bf16 = mybir.dt.bfloat16
    i16 = mybir.dt.int16
    add = mybir.AluOpType.add
    sub = mybir.AluOpType.subtract
    mult = mybir.AluOpType.mult
    Relu = mybir.ActivationFunctionType.Relu

    nc = bacc.Bacc("TRN2", target_bir_lowering=False, debug=False,
                   num_devices=NCORES, num_swdge_queues=NQ)
    utab = nc.dram_tensor("utab", [TROWS, 64], f32, kind="ExternalInput").ap()
    uown2 = nc.dram_tensor("uown2", [P, Q * 128], f32,
                           kind="ExternalInput").ap()
    ilo = nc.dram_tensor("ilo", [128, clo // 16], i16, kind="ExternalInput").ap()
    ihi = nc.dram_tensor("ihi", [128, chi // 16], i16, kind="ExternalInput").ap()
    shi = nc.dram_tensor("shi", [128, SLOTS // 16], i16, kind="ExternalInput").ap()
    wallp = nc.dram_tensor("wall", [128, 320], bf16, kind="ExternalInput").ap()
    w2p = nc.dram_tensor("w2", [64, 128], bf16, kind="ExternalInput").ap()
    bncp = nc.dram_tensor("bnc", [64, 6], f32, kind="ExternalInput").ap()
    b2vp = nc.dram_tensor("b2v", [64, 1], f32, kind="ExternalInput").ap()
    wallfp = nc.dram_tensor("wallf", [128, 320], f32, kind="ExternalInput").ap()
    wallTp = nc.dram_tensor("wallT", [64, 640], f32, kind="ExternalInput").ap()
    o_ap = nc.dram_tensor("o", [256, PC], f32, kind="ExternalOutput").ap()

    # chunks of up to 4 column-groups (512 slots)
    chunks = [(q0, min(4, Q - q0)) for q0 in range(0, Q, 4)]
    pieces_lo, _ = _pieces(list(meta["lo"]), CALL)
    pieces_hi, _ = _pieces(list(meta["hi"]), CALL)
    last_call_q = [0] * Q
    for ci, ps in enumerate(pieces_lo):
        for (_sj, nj, qa) in ps:
            for q in range(qa, min(qa + nj, Q)):
                last_call_q[q] = ci
    ready_at = {}
    for gi, (q0, nq) in enumerate(chunks):
        rc = max(last_call_q[q0:q0 + nq])
        ready_at.setdefault(rc, []).append(gi)

    with tile.TileContext(nc) as tc:
        with (
            tc.tile_pool(name="dram", bufs=1, space="DRAM") as dp,
            tc.tile_pool(name="persist", bufs=1) as mp,
        ):
            accd = dp.tile([SLOTS, 64], f32)

            # ---------- persistent tiles (idx tables first: gathers wait) ----
            it_hi = mp.tile([128, chi // 16], i16, tag="it_hi")
            nc.sync.dma_start(out=it_hi[:], in_=ihi[:])
            it_lo = mp.tile([128, clo // 16], i16, tag="it_lo")
            nc.sync.dma_start(out=it_lo[:], in_=ilo[:])
            st = mp.tile([128, SLOTS // 16], i16, tag="st_hi")
            nc.sync.dma_start(out=st[:], in_=shi[:])
            wt = mp.tile([P, 320], bf16, tag="wt")
            nc.sync.dma_start(out=wt[:], in_=wallp[:])
            wtf = mp.tile([P, 320], f32, tag="wtf")
            nc.sync.dma_start(out=wtf[:], in_=wallfp[:])
            wtT = mp.tile([64, 640], f32, tag="wtT")
            nc.sync.dma_start(out=wtT[:], in_=wallTp[:])
            w2t = mp.tile([64, 128], bf16, tag="w2t")
            nc.sync.dma_start(out=w2t[:], in_=w2p[:])
            bnct = mp.tile([64, 6], f32, tag="bnct")
            nc.sync.dma_start(out=bnct[:], in_=bncp[:])
            b2t = mp.tile([64, 1], f32, tag="b2t")
            nc.sync.dma_start(out=b2t[:], in_=b2vp[:])
            ident = mp.tile([P, P], bf16, tag="ident")
            make_identity(nc, ident[:])
            own2 = mp.tile([P, Q, 128], f32, tag="own2")
            nc.sync.dma_start(out=own2[:].rearrange("p q d -> p (q d)"),
                              in_=uown2[:])

            acc_lo = mp.tile([P, Q, 64], f32, tag="acc_lo")
            nc.vector.memset(acc_lo[:].rearrange("p q d -> p (q d)"), 0.0)
            acc_hi = mp.tile([P, Q, 64], f32, tag="acc_hi")
            nc.vector.memset(acc_hi[:].rearrange("p q d -> p (q d)"), 0.0)
            accm = mp.tile([P, Q, 64], f32, tag="accm")
            csum = mp.tile([P, 1], f32, tag="csum")
            nc.vector.memset(csum[:], 0.0)
            hA = mp.tile([P, SLOTS], bf16, tag="hA")
            hB = mp.tile([P, SLOTS], bf16, tag="hB")
            hC = mp.tile([64, SLOTS], bf16, tag="hC")
            stats = mp.tile([64, 4], f32, tag="stats")
            m2s = mp.tile([P, P], f32, tag="m2s")

            qrot = [0]

            def rr():
                qn = qrot[0] % NQ
                qrot[0] += 1
                return qn

            # ---------- phase 1: gather + ELL accumulate + chunk epilogue ---
            with (
                tc.tile_pool(name="stg", bufs=12) as sp,
                tc.tile_pool(name="zp", bufs=1) as zp,
                tc.tile_pool(name="stkp", bufs=3) as stkp,
                tc.tile_pool(name="ptp", bufs=1, space="PSUM") as ptp,
                tc.tile_pool(name="pw1", bufs=2, space="PSUM") as pw1,
                tc.tile_pool(name="pm2m", bufs=1, space="PSUM") as pm2m,
            ):
                zt = zp.tile([P, Q * 64], f32, tag="zt")
                nc.vector.memset(zt[:], 0.0)
                nc.sync.dma_start(
                    out=accd[:].rearrange("(q p) d -> p q d", p=P),
                    in_=zt[:].rearrange("p (q d) -> p q d", d=64))

                m2 = pm2m.tile([P, P], f32, tag="m2", space="PSUM")
                m2_cnt = [0]

                def emit_chunk(gi):
                    q0, nq = chunks[gi]
                    cw = nq * P
                    c0 = q0 * P
                    stk = stkp.tile([P, 4, 128], f32, tag="stk",
                                    name=f"stk_{gi}")
                    nc.sync.dma_start(
                        out=stk[:, 0:nq, :].rearrange("p q d -> p (q d)"),
                        in_=uown2[:, q0 * 128:(q0 + nq) * 128])
                    # shared msg cols 0:48 (u-agg cols 0:48); wall reads 0:64
                    nc.vector.tensor_tensor(
                        out=stk[:, 0:nq, 0:48], in0=stk[:, 0:nq, 0:48],
                        in1=accm[:, q0:q0 + nq, 0:48], op=add)
                    nc.vector.tensor_tensor(
                        out=stk[:, 0:nq, 0:48], in0=stk[:, 0:nq, 0:48],
                        in1=acc_lo[:, q0:q0 + nq, 0:48], op=add)
                    # agg-path x part cols 64:80
                    nc.vector.tensor_tensor(
                        out=stk[:, 0:nq, 64:80], in0=stk[:, 0:nq, 64:80],
                        in1=accm[:, q0:q0 + nq, 0:16], op=add)
                    nc.vector.tensor_tensor(
                        out=stk[:, 0:nq, 64:80], in0=stk[:, 0:nq, 64:80],
                        in1=acc_lo[:, q0:q0 + nq, 0:16], op=add)
                    # agg-path c part cols 80:88: own + 0.25*sum_s agg_c_s
                    tcc = stkp.tile([P, 4, 32], f32, tag="tcc",
                                    name=f"tcc_{gi}")
                    nc.vector.tensor_tensor(
                        out=tcc[:, 0:nq, :], in0=accm[:, q0:q0 + nq, 16:48],
                        in1=acc_lo[:, q0:q0 + nq, 16:48], op=add)
                    nc.vector.tensor_tensor(
                        out=tcc[:, 0:nq, 0:8], in0=tcc[:, 0:nq, 0:8],
                        in1=tcc[:, 0:nq, 8:16], op=add)
                    nc.vector.tensor_tensor(
                        out=tcc[:, 0:nq, 16:24], in0=tcc[:, 0:nq, 16:24],
                        in1=tcc[:, 0:nq, 24:32], op=add)
                    nc.vector.tensor_tensor(
                        out=tcc[:, 0:nq, 0:8], in0=tcc[:, 0:nq, 0:8],
                        in1=tcc[:, 0:nq, 16:24], op=add)
                    nc.vector.scalar_tensor_tensor(
                        out=stk[:, 0:nq, 80:88], in0=tcc[:, 0:nq, 0:8],
                        scalar=0.25, op0=mult,
                        in1=stk[:, 0:nq, 80:88], op1=add)
                    # bf16 + transpose to feature-major
                    stkb = stkp.tile([P, 4, 128], bf16, tag="stkb",
                                     name=f"stkb_{gi}")
                    nc.scalar.copy(
                        out=stkb[:, 0:nq, :].rearrange("p q d -> p (q d)"),
                        in_=stk[:, 0:nq, :].rearrange("p q d -> p (q d)"))
                    tp = ptp.tile([P, 4, P], bf16, tag="tp", name=f"tp_{gi}",
                                  space="PSUM")
                    for qq in range(nq):
                        nc.tensor.transpose(out=tp[:, qq, :],
                                            in_=stkb[:, qq, :],
                                            identity=ident[:])
                        nc.tensor.matmul(out=m2[:], lhsT=stkb[:, qq, :],
                                         rhs=stkb[:, qq, :],
                                         start=(m2_cnt[0] == 0),
                                         stop=(m2_cnt[0] == Q - 1))
                        m2_cnt[0] += 1
                    stT = stkp.tile([P, 512], bf16, tag="stT",
                                    name=f"stT_{gi}")
                    nc.scalar.copy(
                        out=stT[:, 0:cw],
                        in_=tp[:, 0:nq, :].rearrange("p q d -> p (q d)"))
                    # colsum accumulate (over stacked features)
                    redS = stkp.tile([P, 1], f32, tag="redS",
                                     name=f"redS_{gi}")
                    nc.vector.tensor_reduce(out=redS[:], in_=stT[:, 0:cw],
                                            axis=mybir.AxisListType.X, op=add)
                    nc.vector.tensor_tensor(out=csum[:], in0=csum[:],
                                            in1=redS[:], op=add)
                    # W1 matmuls -> h1 (bf16, no BN yet)
                    psA = pw1.tile([P, 512], f32, tag="psA",
                                   name=f"psA_{gi}", space="PSUM")
                    nc.tensor.matmul(out=psA[:, 0:cw], lhsT=wt[:, 0:128],
                                     rhs=stT[:, 0:cw], start=True, stop=True)
                    nc.scalar.copy(out=hA[:, c0:c0 + cw], in_=psA[:, 0:cw])
                    psB = pw1.tile([P, 512], f32, tag="psB",
                                   name=f"psB_{gi}", space="PSUM")
                    nc.tensor.matmul(out=psB[:, 0:cw], lhsT=wt[:, 128:256],
                                     rhs=stT[:, 0:cw], start=True, stop=True)
                    nc.scalar.copy(out=hB[:, c0:c0 + cw], in_=psB[:, 0:cw])
                    psC = pw1.tile([64, 512], f32, tag="psC",
                                   name=f"psC_{gi}", space="PSUM")
                    nc.tensor.matmul(out=psC[:, 0:cw], lhsT=wt[:, 256:320],
                                     rhs=stT[:, 0:cw], start=True, stop=True)
                    nc.scalar.copy(out=hC[:, c0:c0 + cw], in_=psC[:, 0:cw])

                # --- hi phase: gathers + adds, then scatter-merge ---
                for ci, ps in enumerate(pieces_hi):
                    g = sp.tile([P, CALL // P, 64], f32, tag="stg",
                                name=f"g_hi_{ci}")
                    nc.gpsimd.dma_gather(
                        out_ap=g[:], in_ap=utab[HI_BASE:TROWS, :],
                        idxs_ap=it_hi[:, ci * (CALL // 16):
                                      (ci + 1) * (CALL // 16)],
                        num_idxs=CALL, num_idxs_reg=CALL, elem_size=64,
                        queue_num=rr())
                    for (sj, nj, qa) in ps:
                        nc.vector.tensor_tensor(
                            out=acc_hi[:, qa:qa + nj, :],
                            in0=acc_hi[:, qa:qa + nj, :],
                            in1=g[:, sj:sj + nj, :], op=add)
                for si, c0 in enumerate(range(0, SLOTS, SCALL)):
                    n = min(SCALL, SLOTS - c0)
                    nc.gpsimd.dma_scatter_add(
                        accd[:], acc_hi[:, c0 // P:(c0 + n) // P, :],
                        st[:, c0 // 16:(c0 + n) // 16],
                        num_idxs=n, num_idxs_reg=n, elem_size=64,
                        queue_num=rr())
                nc.sync.dma_start(
                    out=accm[:],
                    in_=accd[:].rearrange("(q p) d -> p q d", p=P))

                # --- lo phase with interleaved chunk work ---
                for ci, ps in enumerate(pieces_lo):
                    g = sp.tile([P, CALL // P, 64], f32, tag="stg",
                                name=f"g_lo_{ci}")
                    nc.gpsimd.dma_gather(
                        out_ap=g[:], in_ap=utab[0:LO_ROWS, :],
                        idxs_ap=it_lo[:, ci * (CALL // 16):
                                      (ci + 1) * (CALL // 16)],
                        num_idxs=CALL, num_idxs_reg=CALL, elem_size=64,
                        queue_num=rr())
                    for (sj, nj, qa) in ps:
                        nc.vector.tensor_tensor(
                            out=acc_lo[:, qa:qa + nj, :],
                            in0=acc_lo[:, qa:qa + nj, :],
                            in1=g[:, sj:sj + nj, :], op=add)
                    for gi in ready_at.get(ci, []):
                        emit_chunk(gi)
                nc.vector.tensor_copy(out=m2s[:], in_=m2[:])

            # ---------- stats fold + AllReduce + BN + pass 2 ----------
            cin = dp.tile([64, 4], f32)
            cout = dp.tile([64, 4], f32)
            with (
                tc.tile_pool(name="po", bufs=3, space="PSUM") as po,
                tc.tile_pool(name="pagp", bufs=1, space="PSUM") as pagp,
                tc.tile_pool(name="pst", bufs=1, space="PSUM") as pst,
                tc.tile_pool(name="rts", bufs=1) as rts,
                tc.tile_pool(name="outs", bufs=4) as osb,
                tc.tile_pool(name="bnp", bufs=1) as bnp,
            ):
                # s1 (sum of z) per 64-block via wall^T @ colsum;
                # s2 (sum of z^2) via w^T M2 w per block
                s1 = bnp.tile([64, 2], f32, tag="s1")
                s2 = bnp.tile([64, 2], f32, tag="s2")
                nc.vector.memset(s1[:], 0.0)
                nc.vector.memset(s2[:], 0.0)
                for b in range(5):
                    col = 0 if b < 4 else 1
                    wm = pst.tile([64, P], f32, tag="wm", name=f"wm_{b}",
                                  space="PSUM")
                    nc.tensor.matmul(out=wm[:], lhsT=wtf[:, b * 64:(b + 1) * 64],
                                     rhs=m2s[:], start=True, stop=True)
                    prod = bnp.tile([64, P], f32, tag="prod", name=f"prod_{b}")
                    nc.vector.tensor_tensor(
                        out=prod[:], in0=wm[:],
                        in1=wtT[:, b * 128:(b + 1) * 128], op=mult)
                    r2 = bnp.tile([64, 1], f32, tag="r2", name=f"r2_{b}")
                    nc.vector.tensor_reduce(out=r2[:], in_=prod[:],
                                            axis=mybir.AxisListType.X, op=add)
                    nc.vector.tensor_tensor(
                        out=s2[:, col:col + 1], in0=s2[:, col:col + 1],
                        in1=r2[:], op=add)
                    p1 = pst.tile([64, 1], f32, tag="p1", name=f"p1_{b}",
                                  space="PSUM")
                    nc.tensor.matmul(out=p1[:], lhsT=wtf[:, b * 64:(b + 1) * 64],
                                     rhs=csum[:], start=True, stop=True)
                    r1 = bnp.tile([64, 1], f32, tag="r1", name=f"r1_{b}")
                    nc.vector.tensor_copy(out=r1[:], in_=p1[:])
                    nc.vector.tensor_tensor(
                        out=s1[:, col:col + 1], in0=s1[:, col:col + 1],
                        in1=r1[:], op=add)
                nc.vector.tensor_copy(out=stats[:, 0:2], in_=s1[:])
                nc.vector.tensor_copy(out=stats[:, 2:4], in_=s2[:])

                nc.sync.dma_start(out=cin[:], in_=stats[:])
                nc.gpsimd.collective_compute(
                    "AllReduce", add,
                    replica_groups=[list(range(NCORES))],
                    ins=[cin.opt()], outs=[cout.opt()])
                nc.sync.dma_start(out=stats[:], in_=cout[:])

                gcols = bnp.tile([64, 2], f32, tag="gcols")
                becols = bnp.tile([64, 2], f32, tag="becols")
                incols = bnp.tile([64, 2], f32, tag="incols")
                nc.vector.tensor_copy(out=gcols[:, 0:1], in_=bnct[:, 0:1])
                nc.vector.tensor_copy(out=gcols[:, 1:2], in_=bnct[:, 3:4])
                nc.vector.tensor_copy(out=becols[:, 0:1], in_=bnct[:, 1:2])
                nc.vector.tensor_copy(out=becols[:, 1:2], in_=bnct[:, 4:5])
                nc.vector.tensor_copy(out=incols[:, 0:1], in_=bnct[:, 2:3])
                nc.vector.tensor_copy(out=incols[:, 1:2], in_=bnct[:, 5:6])
                mu = bnp.tile([64, 2], f32, tag="mu")
                var = bnp.tile([64, 2], f32, tag="var")
                scl = bnp.tile([64, 2], f32, tag="scl")
                bia = bnp.tile([64, 2], f32, tag="bia")
                nc.vector.tensor_tensor(out=mu[:], in0=stats[:, 0:2],
                                        in1=incols[:], op=mult)
                nc.vector.tensor_tensor(out=var[:], in0=stats[:, 2:4],
                                        in1=incols[:], op=mult)
                nc.vector.tensor_tensor(out=scl[:], in0=mu[:], in1=mu[:],
                                        op=mult)
                nc.vector.tensor_tensor(out=var[:], in0=var[:], in1=scl[:],
                                        op=sub)
                nc.vector.tensor_scalar_add(out=var[:], in0=var[:],
                                            scalar1=float(BN_EPS))
                nc.scalar.sqrt(out=var[:], in_=var[:])
                nc.vector.reciprocal(out=var[:], in_=var[:])
                nc.vector.tensor_tensor(out=scl[:], in0=var[:], in1=gcols[:],
                                        op=mult)
                nc.vector.tensor_tensor(out=bia[:], in0=mu[:], in1=scl[:],
                                        op=mult)
                nc.vector.tensor_tensor(out=bia[:], in0=becols[:], in1=bia[:],
                                        op=sub)

                # ---------- pass 2: BN+relu from h1, W2, output ----------
                rC = rts.tile([64, SLOTS], bf16, tag="rt_c", name="rt_c")
                nc.scalar.activation(
                    out=rC[:], in_=hC[:], func=Relu,
                    bias=bia[:, 1:2], scale=scl[:, 1:2])
                rt = {}
                for s in range(S):
                    h = hA if s < 2 else hB
                    sl = slice(0, 64) if s % 2 == 0 else slice(64, 128)
                    r = rts.tile([64, SLOTS], bf16, tag=f"rt_{s}",
                                 name=f"rt_{s}")
                    nc.scalar.activation(
                        out=r[:], in_=h[sl, :], func=Relu,
                        bias=bia[:, 0:1], scale=scl[:, 0:1])
                    rt[s] = r
                PW = 512
                for gi, c0 in enumerate(range(0, PC, PW)):
                    cn = min(PC - c0, PW)
                    pag = pagp.tile([64, PW], f32, tag="pag",
                                    name=f"pag_{gi}", space="PSUM")
                    nc.tensor.matmul(out=pag[:, 0:cn], lhsT=w2t[:, 64:128],
                                     rhs=rC[:, c0:c0 + cn],
                                     start=True, stop=True)
                    aggsb = osb.tile([64, PW], f32, tag="aggsb",
                                     name=f"aggsb_{gi}")
                    nc.vector.tensor_tensor(
                        out=aggsb[:, 0:cn], in0=pag[:, 0:cn],
                        in1=b2t[:, 0:1].to_broadcast([64, cn]), op=add)
                    for s in range(S):
                        pso = po.tile([64, PW], f32, tag="pso",
                                      name=f"pso_{gi}_{s}", space="PSUM")
                        nc.tensor.matmul(out=pso[:, 0:cn], lhsT=w2t[:, 0:64],
                                         rhs=rt[s][:, c0:c0 + cn],
                                         start=True, stop=True)
                        ot = osb.tile([64, PW], f32, tag="ot",
                                      name=f"ot_{gi}_{s}")
                        nc.vector.tensor_tensor(
                            out=ot[:, 0:cn], in0=pso[:, 0:cn],
                            in1=aggsb[:, 0:cn], op=add)
                        nc.sync.dma_start(
                            out=o_ap[s * 64:(s + 1) * 64, c0:c0 + cn],
                            in_=ot[:, 0:cn])
    # Align each gather/scatter's SWDGE queue with its post-schedule DMASW
    # sem lane (8 lanes round-robin over Pool DMA insts): lane % NQ.  The
    # scheduler reorders Pool DMA insts, so emission-order rotation would
    # put two queues on one sem lane (rejected by ucode ring metadata).
    import concourse.mybir as _mb
    idx = 0
    for blk in nc.m.functions[0].blocks:
        for inst in blk.instructions:
            tn = type(inst).__name__
            if tn in ("InstDMAGatherAnt", "InstDMAScatterAddAnt"):
                inst.queue_num = (idx % 8) % NQ
                idx += 1
            elif tn == "InstDMACopy" and inst.engine == _mb.EngineType.Pool:
                raise AssertionError("unexpected Pool-engine DMACopy")
    nc.compile()
    return nc


def kernel(**inputs):
    global last_exec_time_ns
    from concourse import bass_utils

    x = np.asarray(inputs["x"], np.float32)
    c = np.asarray(inputs["c"], np.float32)
    edge_index = np.asarray(inputs["edge_index"])
    eps_s = float(np.asarray(inputs["eps_shared"]))
    eps_a = float(np.asarray(inputs["eps_agg"]))
    args = [np.asarray(inputs[k], np.float32) for k in
            ("W1s", "g1s", "be1s", "W2s", "b2s",
             "W1a", "g1a", "be1a", "W2a", "b2a")]

    ret_maps, pre_maps, meta, orders = _build_inputs(
        x, c, edge_index, eps_a, *args)
    in_maps = _finish_inputs(ret_maps, pre_maps, eps_s, eps_a)
    clo = in_maps[0]["ilo"].shape[1] * 16
    chi = in_maps[0]["ihi"].shape[1] * 16
    key = (meta["lo"], meta["hi"], eps_s, eps_a, clo, chi)
    if key not in _prog_cache:
        _prog_cache[key] = _build_program(meta, eps_s, eps_a, clo, chi)
    nc = _prog_cache[key]

    kwargs = {}
    if _trace:
        try:
            import axon_profile_shim  # noqa: F401
        except ImportError:
            pass
        kwargs["trace"] = True
    res = bass_utils.run_bass_kernel_spmd(
        nc, in_maps, core_ids=list(range(NCORES)), **kwargs)
    last_exec_time_ns = res.exec_time_ns

    out = np.empty((N, S * EMB), dtype=np.float32)
    for k in range(NCORES):
        ok = res.results[k]["o"]          # [256, PC]
        out[k * PC + orders[k]] = ok.T
    return out
